# revision 23
# baseline (speedup 1.0000x reference)
"""Causal self-attention Trainium2 kernel.

Problem: B=4, T=2048, D=1024, H=16 heads (hd=64).
Sharding: 8 cores; core c -> batch c//2, heads (c%2)*8 .. +8.
Each core computes a partial output projection (its 512 rows of w_proj);
host sums the two partials per batch and adds b_proj.

Layout strategy (per core):
  - x^T [D, T] streamed in fp32, consumed as float32r (full-rate matmuls
    at near-fp32 precision for the QKV projections; host pre-transposed).
  - Q^T, K^T computed as [512, 2048] (head-dim on partitions) via
    W-stationary matmuls: out = W_chunk.T @ x^T, stored bf16.
  - V computed in natural [T, 512] layout (x^T-stationary), stored per-head
    augmented with a ones column -> [128k, head, 65], so the P@V matmul
    accumulates softmax denominators for free in row 64.
  - Scores computed transposed: S^T[k, q] = (K^T_chunk).T @ Q^T, causal
    blocks only; exp on ScalarE straight out of PSUM (no max subtraction --
    scaled scores are ~N(0,1), max << 88); triangular mask multiply only on
    diagonal 128-blocks.
  - P@V with V_aug stationary: out^T[65, q] accumulated over k-chunks in
    PSUM. Row 64 = sum of exp. Normalize with DVE reciprocal + GpSimd
    partition_broadcast; result written as A^T [512, 2048] bf16 which is
    exactly the lhsT needed for the output projection.
"""

import sys

for _p in ("/opt/trn_rl_repo",):
    if _p not in sys.path:
        sys.path.insert(0, _p)

import numpy as np
import ml_dtypes

import concourse.bass as bass
import concourse.mybir as mybir
import concourse.tile as tile
from concourse import bacc
from concourse.bass_utils import run_bass_kernel_spmd

BF16 = ml_dtypes.bfloat16

B, T, D = 4, 2048, 1024
H, HD = 16, 64
NCORES = 8
HPC = 8                  # heads per core
GCOLS = HPC * HD         # 512 columns of qkv per core per q/k/v
P = 128
NDC = D // P             # 8 contraction chunks of 128
NTT = T // P             # 16 t-tiles of 128
NQC = T // 512           # 4 q-chunks of 512
NMC = GCOLS // P         # 4 M-chunks per Q^T / K^T


def build_nc(trace_sim: bool = False):
    f32 = mybir.dt.float32
    f32r = mybir.dt.float32r
    bf16 = mybir.dt.bfloat16

    nc = bacc.Bacc("TRN2", target_bir_lowering=False, debug=False,
                   num_devices=NCORES)

    xT_d = nc.dram_tensor("xT", [D, T], f32r, kind="ExternalInput")
    wqk_d = nc.dram_tensor("wqk", [D, 2 * GCOLS], f32r, kind="ExternalInput")
    wv_d = nc.dram_tensor("wv", [D, GCOLS], f32r, kind="ExternalInput")
    wp_d = nc.dram_tensor("wp", [GCOLS, D], bf16, kind="ExternalInput")
    bqk_d = nc.dram_tensor("bqk", [P, 2 * NMC], f32, kind="ExternalInput")
    bv_d = nc.dram_tensor("bv", [GCOLS], f32, kind="ExternalInput")
    tri_d = nc.dram_tensor("tri", [P, P], bf16, kind="ExternalInput")
    out_d = nc.dram_tensor("outp", [T, D], f32, kind="ExternalOutput")

    with tile.TileContext(nc, trace_sim=trace_sim) as tc:
        with (
            tc.tile_pool(name="consts", bufs=1) as consts,
            tc.tile_pool(name="weights", bufs=1) as weights,
            tc.tile_pool(name="acts", bufs=1) as acts,
            tc.tile_pool(name="pt", bufs=3) as ptp,
            tc.tile_pool(name="norm", bufs=2) as normp,
            tc.tile_pool(name="outs", bufs=3) as outsp,
            tc.tile_pool(name="ps_mm", bufs=2, space="PSUM") as ps_mm,
            tc.tile_pool(name="ps_st", bufs=2, space="PSUM") as ps_st,
            tc.tile_pool(name="ps_o", bufs=2, space="PSUM") as ps_o,
        ):
            # ---------------- constants / weights ----------------
            tri_sb = consts.tile([P, P], bf16)
            nc.sync.dma_start(tri_sb[:], tri_d.ap())
            bqk_sb = consts.tile([P, 2 * NMC], f32)
            nc.sync.dma_start(bqk_sb[:], bqk_d.ap())
            # b_v replicated to all partitions (varies along free dim)
            bv_rep = consts.tile([P, GCOLS], f32)
            bv_ap = bv_d.ap()
            nc.gpsimd.dma_start(
                bv_rep[:],
                bass.AP(tensor=bv_ap.tensor, offset=bv_ap.offset,
                        ap=[[0, P]] + list(bv_ap.ap)),
            )

            # DMA order = first-use order: wv + x^T piece 0 feed the V and
            # mch-0 projections; then wqk, the remaining x^T pieces, wp.
            wv_sb = weights.tile([P, NDC, GCOLS], f32r)
            xT_sb = acts.tile([P, NDC, T], f32r)
            for dc in range(NDC):
                nc.sync.dma_start(
                    xT_sb[:, dc, 0:512], xT_d[dc * P:(dc + 1) * P, 0:512])
                nc.sync.dma_start(wv_sb[:, dc, :], wv_d[dc * P:(dc + 1) * P, :])
            # mch-0's K/Q weight columns first (K0 = cols 512:640,
            # Q0 = cols 0:128), so the first attention chunk isn't gated
            # on the full wqk transfer.
            wqk_sb = weights.tile([P, NDC, 2 * GCOLS], f32r)
            for dc in range(NDC):
                nc.sync.dma_start(wqk_sb[:, dc, GCOLS:GCOLS + P],
                                  wqk_d[dc * P:(dc + 1) * P, GCOLS:GCOLS + P])
                nc.sync.dma_start(wqk_sb[:, dc, 0:P],
                                  wqk_d[dc * P:(dc + 1) * P, 0:P])
            for cp in range(1, NQC):
                for dc in range(NDC):
                    nc.sync.dma_start(
                        xT_sb[:, dc, cp * 512:(cp + 1) * 512],
                        xT_d[dc * P:(dc + 1) * P, cp * 512:(cp + 1) * 512])
            for dc in range(NDC):
                nc.sync.dma_start(wqk_sb[:, dc, P:GCOLS],
                                  wqk_d[dc * P:(dc + 1) * P, P:GCOLS])
                nc.sync.dma_start(wqk_sb[:, dc, GCOLS + P:],
                                  wqk_d[dc * P:(dc + 1) * P, GCOLS + P:])
            wp_sb = weights.tile([P, NMC, D], bf16)
            for hc in range(NMC):
                nc.sync.dma_start(wp_sb[:, hc, :], wp_d[hc * P:(hc + 1) * P, :])

            # ---------------- phases 1+2 interleaved ----------------
            # warm the ScalarE Exp table during the startup DMA window so
            # the first attention block doesn't pay the table load
            warm = consts.tile([1, 1], f32)
            nc.vector.memset(warm[:], 0.0)
            nc.scalar.activation(warm[:], warm[:],
                                 mybir.ActivationFunctionType.Exp)

            # V natural + ones column: [128, tt, head, 65]
            V_sb = acts.tile([P, NTT, HPC, HD + 1], bf16)
            nc.vector.memset(V_sb[:, :, :, HD], 1.0)

            def project_v(tts):
                for tt in tts:
                    pv = ps_mm.tile([P, 512], f32, tag="mm")
                    for dc in range(NDC):
                        nc.tensor.matmul(
                            pv[:],
                            xT_sb[:, dc, tt * P:(tt + 1) * P],
                            wv_sb[:, dc, :],
                            start=(dc == 0), stop=(dc == NDC - 1),
                        )
                    nc.vector.tensor_tensor(
                        V_sb[:, tt, :, 0:HD],
                        pv[:].rearrange("p (h d) -> p h d", h=HPC),
                        bv_rep[:].rearrange("p (h d) -> p h d", h=HPC),
                        mybir.AluOpType.add,
                    )

            # Q^T / K^T / A^T: [512, T] each, stored as [128, chunk, T].
            QT_sb = acts.tile([P, NMC, T], bf16)
            KT_sb = acts.tile([P, NMC, T], bf16)
            AT_sb = acts.tile([P, NMC, T], bf16)

            def project_qk(m, tc4s=range(NQC)):
                for tc4 in tc4s:
                    pq = ps_mm.tile([P, 512], f32, tag="mm")
                    for dc in range(NDC):
                        nc.tensor.matmul(
                            pq[:],
                            wqk_sb[:, dc, m * P:(m + 1) * P],
                            xT_sb[:, dc, tc4 * 512:(tc4 + 1) * 512],
                            start=(dc == 0), stop=(dc == NDC - 1),
                        )
                    dst = (QT_sb if m < NMC else KT_sb)
                    nc.vector.tensor_scalar_add(
                        dst[:, m % NMC, tc4 * 512:(tc4 + 1) * 512],
                        pq[:], bqk_sb[:, m:m + 1],
                    )

            def project_out(tts):
                for tt in tts:
                    for ncol in range(2):
                        pp = ps_mm.tile([P, 512], f32, tag="mm")
                        for hc in range(NMC):
                            nc.tensor.matmul(
                                pp[:],
                                AT_sb[:, hc, tt * P:(tt + 1) * P],
                                wp_sb[:, hc, ncol * 512:(ncol + 1) * 512],
                                start=(hc == 0), stop=(hc == NMC - 1),
                            )
                        ot = outsp.tile([P, 512], f32, tag="ot")
                        nc.vector.tensor_copy(ot[:], pp[:])
                        nc.sync.dma_start(
                            out_d[tt * P:(tt + 1) * P,
                                  ncol * 512:(ncol + 1) * 512],
                            ot[:],
                        )

            # Per 128-chunk: project K then Q, then both heads' attention.
            # The head pair sits at partitions 0-63 / 64-127, so the two
            # K=64 score matmuls auto-derive tile_position (0,0)/(64,0)
            # and can run concurrently on the two PE array row-halves.
            # V projection is smeared across mch 0's qc blocks (only V
            # k-tiles <= 4qc+3 are needed there); the output projection is
            # smeared across mch 3's qc blocks (proj of t-range qc needs
            # every head's qc block done).
            project_v(range(0, 2))
            project_qk(NMC, [0, 1])     # K chunk 0 (first half)
            project_v(range(2, 4))
            project_qk(NMC, [2, 3])
            project_qk(0)               # Q chunk 0
            for mch in range(NMC):
                for qc in range(NQC):
                    po = [ps_o.tile([HD + 1, 512], f32, tag="po",
                                    name=f"po{j}")
                          for j in range(2)]
                    nki = 4 * qc + 4
                    for ki in range(nki):
                        off = max(0, ki - 4 * qc) * P
                        # head-pair S^T into one 2-bank psum tile; one exp
                        # instruction covers both heads (halves the per-op
                        # PSUM-access overhead on ScalarE).
                        pshat = ps_st.tile([P, 2, 512], f32, tag="st")
                        pts = ptp.tile([P, 2, 512], bf16, tag="pt")
                        for j in range(2):
                            part = j * 64
                            nc.tensor.matmul(
                                pshat[:, j, off:512],
                                KT_sb[part:part + 64, mch,
                                      ki * P:(ki + 1) * P],
                                QT_sb[part:part + 64, mch,
                                      qc * 512 + off:(qc + 1) * 512],
                                start=True, stop=True,
                            )
                        nc.scalar.activation(
                            pts[:, :, off:512], pshat[:, :, off:512],
                            mybir.ActivationFunctionType.Exp,
                            scale=0.125,
                        )
                        if ki >= 4 * qc:
                            # diagonal block: zero out q < k entries
                            for j in range(2):
                                nc.vector.tensor_tensor(
                                    pts[:, j, off:off + P],
                                    pts[:, j, off:off + P],
                                    tri_sb[:], mybir.AluOpType.mult,
                                )
                        for j in range(2):
                            nc.tensor.matmul(
                                po[j][:, off:512],
                                V_sb[:, ki, 2 * mch + j, :],
                                pts[:, j, off:512],
                                start=(ki == 0), stop=(ki == nki - 1),
                            )
                    # normalize: row 64 of po is the softmax denominator.
                    # Copy the unnormalized block out first so the PSUM
                    # slot frees after two quick DVE ops; the broadcast +
                    # in-place multiply run off the critical path.
                    for j in range(2):
                        part = j * 64
                        rs = normp.tile([1, 512], f32, tag="rs")
                        nc.vector.reciprocal(rs[:], po[j][HD:HD + 1, :])
                        rep = normp.tile([64, 512], f32, tag="rep")
                        nc.gpsimd.partition_broadcast(rep[:], rs[0:1, :])
                        nc.vector.tensor_tensor(
                            AT_sb[part:part + 64, mch,
                                  qc * 512:(qc + 1) * 512],
                            po[j][0:HD, :], rep[:], mybir.AluOpType.mult,
                        )
                    if mch == 0 and qc < NQC - 1:
                        project_v(range(4 * (qc + 1), 4 * (qc + 2)))
                    if mch < NMC - 1:
                        # pipeline next chunk's K/Q projection (2 of the 8
                        # 512-col groups per qc block)
                        m_next = [NMC + mch + 1, mch + 1][qc // 2]
                        project_qk(m_next, [2 * qc % 4, 2 * qc % 4 + 1])
                    else:
                        project_out(range(4 * qc, 4 * (qc + 1)))

    nc.compile()
    return nc


def host_inputs(x, w_qkv, b_qkv):
    """Per-core input maps. Core c -> batch c//2, head group c%2."""
    x = np.asarray(x, np.float32)
    w_qkv = np.asarray(w_qkv, np.float32)
    b_qkv = np.asarray(b_qkv, np.float32)
    tri = (np.arange(P)[None, :] >= np.arange(P)[:, None]).astype(BF16)
    in_maps = []
    for c in range(NCORES):
        b, g = c // 2, c % 2
        cs = slice(g * GCOLS, (g + 1) * GCOLS)
        xT = np.ascontiguousarray(x[b].T).astype(np.float32)
        wqk = np.concatenate(
            [w_qkv[:, cs], w_qkv[:, D + g * GCOLS: D + (g + 1) * GCOLS]],
            axis=1).astype(np.float32)
        wv = np.ascontiguousarray(
            w_qkv[:, 2 * D + g * GCOLS: 2 * D + (g + 1) * GCOLS]).astype(np.float32)
        bq = b_qkv[cs].reshape(NMC, P).T
        bk = b_qkv[D + g * GCOLS: D + (g + 1) * GCOLS].reshape(NMC, P).T
        bqk = np.ascontiguousarray(
            np.concatenate([bq, bk], axis=1)).astype(np.float32)
        bv = np.ascontiguousarray(
            b_qkv[2 * D + g * GCOLS: 2 * D + (g + 1) * GCOLS]).astype(np.float32)
        in_maps.append({
            "xT": xT, "wqk": wqk, "wv": wv,
            "wp": None,  # filled by caller (needs w_proj)
            "bqk": bqk, "bv": bv, "tri": tri,
        })
    return in_maps


def full_in_maps(x, w_qkv, b_qkv, w_proj):
    w_proj = np.asarray(w_proj, np.float32)
    in_maps = host_inputs(x, w_qkv, b_qkv)
    for c in range(NCORES):
        g = c % 2
        in_maps[c]["wp"] = np.ascontiguousarray(
            w_proj[g * GCOLS:(g + 1) * GCOLS, :]).astype(BF16)
    return in_maps


def gather(results, b_proj):
    out = np.zeros((B, T, D), np.float32)
    for c in range(NCORES):
        out[c // 2] += results[c]["outp"]
    out += np.asarray(b_proj, np.float32)[None, None, :]
    return out


_NC_CACHE = None


def kernel(x, w_qkv, b_qkv, w_proj, b_proj):
    global _NC_CACHE
    if _NC_CACHE is None:
        _NC_CACHE = build_nc()
    in_maps = full_in_maps(x, w_qkv, b_qkv, w_proj)
    res = run_bass_kernel_spmd(_NC_CACHE, in_maps, core_ids=list(range(NCORES)))
    return gather(res.results, b_proj)


if __name__ == "__main__":
    rng = np.random.default_rng(0)
    x = rng.standard_normal((B, T, D), dtype=np.float32)
    w_qkv = rng.standard_normal((D, 3 * D), dtype=np.float32) / np.sqrt(D)
    b_qkv = np.zeros(3 * D, np.float32)
    w_proj = rng.standard_normal((D, D), dtype=np.float32) / np.sqrt(D)
    b_proj = np.zeros(D, np.float32)
    out = kernel(x, w_qkv, b_qkv, w_proj, b_proj)
    print(out.shape, out.dtype)



# revision 26
# speedup vs baseline: 1.0152x; 1.0152x over previous
"""Causal self-attention Trainium2 kernel.

Problem: B=4, T=2048, D=1024, H=16 heads (hd=64).
Sharding: 8 cores; core c -> batch c//2, heads (c%2)*8 .. +8.
Each core computes a partial output projection (its 512 rows of w_proj);
host sums the two partials per batch and adds b_proj.

Layout strategy (per core):
  - x^T [D, T] streamed in fp32, consumed as float32r (full-rate matmuls
    at near-fp32 precision for the QKV projections; host pre-transposed).
  - Q^T, K^T computed as [512, 2048] (head-dim on partitions) via
    W-stationary matmuls: out = W_chunk.T @ x^T, stored bf16.
  - V computed in natural [T, 512] layout (x^T-stationary), stored per-head
    augmented with a ones column -> [128k, head, 65], so the P@V matmul
    accumulates softmax denominators for free in row 64.
  - Scores computed transposed: S^T[k, q] = (K^T_chunk).T @ Q^T, causal
    blocks only; exp on ScalarE straight out of PSUM (no max subtraction --
    scaled scores are ~N(0,1), max << 88); triangular mask multiply only on
    diagonal 128-blocks.
  - P@V with V_aug stationary: out^T[65, q] accumulated over k-chunks in
    PSUM. Row 64 = sum of exp. Normalize with DVE reciprocal + GpSimd
    partition_broadcast; result written as A^T [512, 2048] bf16 which is
    exactly the lhsT needed for the output projection.
"""

import sys

for _p in ("/opt/trn_rl_repo",):
    if _p not in sys.path:
        sys.path.insert(0, _p)

import numpy as np
import ml_dtypes

import concourse.bass as bass
import concourse.mybir as mybir
import concourse.tile as tile
from concourse import bacc
from concourse.bass_utils import run_bass_kernel_spmd

BF16 = ml_dtypes.bfloat16

B, T, D = 4, 2048, 1024
H, HD = 16, 64
NCORES = 8
HPC = 8                  # heads per core
GCOLS = HPC * HD         # 512 columns of qkv per core per q/k/v
P = 128
NDC = D // P             # 8 contraction chunks of 128
NTT = T // P             # 16 t-tiles of 128
NQC = T // 512           # 4 q-chunks of 512
NMC = GCOLS // P         # 4 M-chunks per Q^T / K^T


def build_nc(trace_sim: bool = False):
    f32 = mybir.dt.float32
    f32r = mybir.dt.float32r
    bf16 = mybir.dt.bfloat16

    nc = bacc.Bacc("TRN2", target_bir_lowering=False, debug=False,
                   num_devices=NCORES)

    xT_d = nc.dram_tensor("xT", [D, T], f32r, kind="ExternalInput")
    wqk_d = nc.dram_tensor("wqk", [D, 2 * GCOLS], f32r, kind="ExternalInput")
    wv_d = nc.dram_tensor("wv", [D, GCOLS], f32r, kind="ExternalInput")
    wp_d = nc.dram_tensor("wp", [GCOLS, D], bf16, kind="ExternalInput")
    bqk_d = nc.dram_tensor("bqk", [P, 2 * NMC], f32, kind="ExternalInput")
    bv_d = nc.dram_tensor("bv", [GCOLS], f32, kind="ExternalInput")
    tri_d = nc.dram_tensor("tri", [P, P], bf16, kind="ExternalInput")
    out_d = nc.dram_tensor("outp", [T, D], f32, kind="ExternalOutput")

    with tile.TileContext(nc, trace_sim=trace_sim) as tc:
        with (
            tc.tile_pool(name="consts", bufs=1) as consts,
            tc.tile_pool(name="weights", bufs=1) as weights,
            tc.tile_pool(name="acts", bufs=1) as acts,
            tc.tile_pool(name="pt", bufs=3) as ptp,
            tc.tile_pool(name="norm", bufs=2) as normp,
            tc.tile_pool(name="outs", bufs=3) as outsp,
            tc.tile_pool(name="ps_mm", bufs=2, space="PSUM") as ps_mm,
            tc.tile_pool(name="ps_st", bufs=2, space="PSUM") as ps_st,
            tc.tile_pool(name="ps_o", bufs=2, space="PSUM") as ps_o,
        ):
            # ---------------- constants / weights ----------------
            tri_sb = consts.tile([P, P], bf16)
            nc.sync.dma_start(tri_sb[:], tri_d.ap())
            bqk_sb = consts.tile([P, 2 * NMC], f32)
            nc.sync.dma_start(bqk_sb[:], bqk_d.ap())
            # b_v replicated to all partitions (varies along free dim)
            bv_rep = consts.tile([P, GCOLS], f32)
            bv_ap = bv_d.ap()
            nc.gpsimd.dma_start(
                bv_rep[:],
                bass.AP(tensor=bv_ap.tensor, offset=bv_ap.offset,
                        ap=[[0, P]] + list(bv_ap.ap)),
            )

            # DMA order = first-use order: wv + x^T piece 0 feed the V and
            # mch-0 projections; then wqk, the remaining x^T pieces, wp.
            # First bytes on the wire: x^T piece 0 plus mch-0's K/Q weight
            # columns (K0 = cols 512:640, Q0 = cols 0:128) -- that is all the
            # first score matmuls need. wv follows for the P@V drain.
            wv_sb = weights.tile([P, NDC, GCOLS], f32r)
            xT_sb = acts.tile([P, NDC, T], f32r)
            wqk_sb = weights.tile([P, NDC, 2 * GCOLS], f32r)
            for dc in range(NDC):
                nc.sync.dma_start(
                    xT_sb[:, dc, 0:512], xT_d[dc * P:(dc + 1) * P, 0:512])
                nc.sync.dma_start(wqk_sb[:, dc, GCOLS:GCOLS + P],
                                  wqk_d[dc * P:(dc + 1) * P, GCOLS:GCOLS + P])
                nc.sync.dma_start(wqk_sb[:, dc, 0:P],
                                  wqk_d[dc * P:(dc + 1) * P, 0:P])
            for dc in range(NDC):
                nc.sync.dma_start(wv_sb[:, dc, :], wv_d[dc * P:(dc + 1) * P, :])
            for cp in range(1, NQC):
                for dc in range(NDC):
                    nc.sync.dma_start(
                        xT_sb[:, dc, cp * 512:(cp + 1) * 512],
                        xT_d[dc * P:(dc + 1) * P, cp * 512:(cp + 1) * 512])
            for dc in range(NDC):
                nc.sync.dma_start(wqk_sb[:, dc, P:GCOLS],
                                  wqk_d[dc * P:(dc + 1) * P, P:GCOLS])
                nc.sync.dma_start(wqk_sb[:, dc, GCOLS + P:],
                                  wqk_d[dc * P:(dc + 1) * P, GCOLS + P:])
            wp_sb = weights.tile([P, NMC, D], bf16)
            for hc in range(NMC):
                nc.sync.dma_start(wp_sb[:, hc, :], wp_d[hc * P:(hc + 1) * P, :])

            # ---------------- phases 1+2 interleaved ----------------
            # warm the ScalarE Exp table during the startup DMA window so
            # the first attention block doesn't pay the table load
            warm = consts.tile([1, 1], f32)
            nc.vector.memset(warm[:], 0.0)
            nc.scalar.activation(warm[:], warm[:],
                                 mybir.ActivationFunctionType.Exp)

            # V natural + ones column: [128, tt, head, 65]
            V_sb = acts.tile([P, NTT, HPC, HD + 1], bf16)
            nc.vector.memset(V_sb[:, :, :, HD], 1.0)

            def project_v(tts):
                for tt in tts:
                    pv = ps_mm.tile([P, 512], f32, tag="mm")
                    for dc in range(NDC):
                        nc.tensor.matmul(
                            pv[:],
                            xT_sb[:, dc, tt * P:(tt + 1) * P],
                            wv_sb[:, dc, :],
                            start=(dc == 0), stop=(dc == NDC - 1),
                        )
                    nc.vector.tensor_tensor(
                        V_sb[:, tt, :, 0:HD],
                        pv[:].rearrange("p (h d) -> p h d", h=HPC),
                        bv_rep[:].rearrange("p (h d) -> p h d", h=HPC),
                        mybir.AluOpType.add,
                    )

            # Q^T / K^T / A^T: [512, T] each, stored as [128, chunk, T].
            QT_sb = acts.tile([P, NMC, T], bf16)
            KT_sb = acts.tile([P, NMC, T], bf16)
            AT_sb = acts.tile([P, NMC, T], bf16)

            def project_qk(m, tc4s=range(NQC)):
                for tc4 in tc4s:
                    pq = ps_mm.tile([P, 512], f32, tag="mm")
                    for dc in range(NDC):
                        nc.tensor.matmul(
                            pq[:],
                            wqk_sb[:, dc, m * P:(m + 1) * P],
                            xT_sb[:, dc, tc4 * 512:(tc4 + 1) * 512],
                            start=(dc == 0), stop=(dc == NDC - 1),
                        )
                    dst = (QT_sb if m < NMC else KT_sb)
                    nc.vector.tensor_scalar_add(
                        dst[:, m % NMC, tc4 * 512:(tc4 + 1) * 512],
                        pq[:], bqk_sb[:, m:m + 1],
                    )

            def project_out(tts):
                for tt in tts:
                    for ncol in range(2):
                        pp = ps_mm.tile([P, 512], f32, tag="mm")
                        for hc in range(NMC):
                            nc.tensor.matmul(
                                pp[:],
                                AT_sb[:, hc, tt * P:(tt + 1) * P],
                                wp_sb[:, hc, ncol * 512:(ncol + 1) * 512],
                                start=(hc == 0), stop=(hc == NMC - 1),
                            )
                        ot = outsp.tile([P, 512], f32, tag="ot")
                        nc.vector.tensor_copy(ot[:], pp[:])
                        nc.sync.dma_start(
                            out_d[tt * P:(tt + 1) * P,
                                  ncol * 512:(ncol + 1) * 512],
                            ot[:],
                        )

            # Per 128-chunk: project K then Q, then both heads' attention.
            # The head pair sits at partitions 0-63 / 64-127, so the two
            # K=64 score matmuls auto-derive tile_position (0,0)/(64,0)
            # and can run concurrently on the two PE array row-halves.
            # V projection is smeared across mch 0's qc blocks (only V
            # k-tiles <= 4qc+3 are needed there); the output projection is
            # smeared across mch 3's qc blocks (proj of t-range qc needs
            # every head's qc block done).
            project_qk(NMC, [0])        # K chunk 0, columns for qc 0
            project_qk(0, [0])          # Q chunk 0, columns for qc 0
            project_v(range(0, 4))
            project_qk(NMC, [1, 2, 3])
            project_qk(0, [1, 2, 3])
            for mch in range(NMC):
                for qc in range(NQC):
                    po = [ps_o.tile([HD + 1, 512], f32, tag="po",
                                    name=f"po{j}")
                          for j in range(2)]
                    nki = 4 * qc + 4
                    for ki in range(nki):
                        off = max(0, ki - 4 * qc) * P
                        # head-pair S^T into one 2-bank psum tile; one exp
                        # instruction covers both heads (halves the per-op
                        # PSUM-access overhead on ScalarE).
                        pshat = ps_st.tile([P, 2, 512], f32, tag="st")
                        pts = ptp.tile([P, 2, 512], bf16, tag="pt")
                        for j in range(2):
                            part = j * 64
                            nc.tensor.matmul(
                                pshat[:, j, off:512],
                                KT_sb[part:part + 64, mch,
                                      ki * P:(ki + 1) * P],
                                QT_sb[part:part + 64, mch,
                                      qc * 512 + off:(qc + 1) * 512],
                                start=True, stop=True,
                            )
                        nc.scalar.activation(
                            pts[:, :, off:512], pshat[:, :, off:512],
                            mybir.ActivationFunctionType.Exp,
                            scale=0.125,
                        )
                        if ki >= 4 * qc:
                            # diagonal block: zero out q < k entries
                            for j in range(2):
                                nc.vector.tensor_tensor(
                                    pts[:, j, off:off + P],
                                    pts[:, j, off:off + P],
                                    tri_sb[:], mybir.AluOpType.mult,
                                )
                        for j in range(2):
                            nc.tensor.matmul(
                                po[j][:, off:512],
                                V_sb[:, ki, 2 * mch + j, :],
                                pts[:, j, off:512],
                                start=(ki == 0), stop=(ki == nki - 1),
                            )
                    # normalize: row 64 of po is the softmax denominator.
                    # Copy the unnormalized block out first so the PSUM
                    # slot frees after two quick DVE ops; the broadcast +
                    # in-place multiply run off the critical path.
                    for j in range(2):
                        part = j * 64
                        rs = normp.tile([1, 512], f32, tag="rs")
                        nc.vector.reciprocal(rs[:], po[j][HD:HD + 1, :])
                        rep = normp.tile([64, 512], f32, tag="rep")
                        nc.gpsimd.partition_broadcast(rep[:], rs[0:1, :])
                        nc.vector.tensor_tensor(
                            AT_sb[part:part + 64, mch,
                                  qc * 512:(qc + 1) * 512],
                            po[j][0:HD, :], rep[:], mybir.AluOpType.mult,
                        )
                    if mch == 0 and qc < NQC - 1:
                        project_v(range(4 * (qc + 1), 4 * (qc + 2)))
                    if mch < NMC - 1:
                        # pipeline next chunk's K/Q projection (2 of the 8
                        # 512-col groups per qc block)
                        m_next = [NMC + mch + 1, mch + 1][qc // 2]
                        project_qk(m_next, [2 * qc % 4, 2 * qc % 4 + 1])
                    else:
                        project_out(range(4 * qc, 4 * (qc + 1)))

    nc.compile()
    return nc


def host_inputs(x, w_qkv, b_qkv):
    """Per-core input maps. Core c -> batch c//2, head group c%2."""
    x = np.asarray(x, np.float32)
    w_qkv = np.asarray(w_qkv, np.float32)
    b_qkv = np.asarray(b_qkv, np.float32)
    tri = (np.arange(P)[None, :] >= np.arange(P)[:, None]).astype(BF16)
    in_maps = []
    for c in range(NCORES):
        b, g = c // 2, c % 2
        cs = slice(g * GCOLS, (g + 1) * GCOLS)
        xT = np.ascontiguousarray(x[b].T).astype(np.float32)
        wqk = np.concatenate(
            [w_qkv[:, cs], w_qkv[:, D + g * GCOLS: D + (g + 1) * GCOLS]],
            axis=1).astype(np.float32)
        wv = np.ascontiguousarray(
            w_qkv[:, 2 * D + g * GCOLS: 2 * D + (g + 1) * GCOLS]).astype(np.float32)
        bq = b_qkv[cs].reshape(NMC, P).T
        bk = b_qkv[D + g * GCOLS: D + (g + 1) * GCOLS].reshape(NMC, P).T
        bqk = np.ascontiguousarray(
            np.concatenate([bq, bk], axis=1)).astype(np.float32)
        bv = np.ascontiguousarray(
            b_qkv[2 * D + g * GCOLS: 2 * D + (g + 1) * GCOLS]).astype(np.float32)
        in_maps.append({
            "xT": xT, "wqk": wqk, "wv": wv,
            "wp": None,  # filled by caller (needs w_proj)
            "bqk": bqk, "bv": bv, "tri": tri,
        })
    return in_maps


def full_in_maps(x, w_qkv, b_qkv, w_proj):
    w_proj = np.asarray(w_proj, np.float32)
    in_maps = host_inputs(x, w_qkv, b_qkv)
    for c in range(NCORES):
        g = c % 2
        in_maps[c]["wp"] = np.ascontiguousarray(
            w_proj[g * GCOLS:(g + 1) * GCOLS, :]).astype(BF16)
    return in_maps


def gather(results, b_proj):
    out = np.zeros((B, T, D), np.float32)
    for c in range(NCORES):
        out[c // 2] += results[c]["outp"]
    out += np.asarray(b_proj, np.float32)[None, None, :]
    return out


_NC_CACHE = None


def kernel(x, w_qkv, b_qkv, w_proj, b_proj):
    global _NC_CACHE
    if _NC_CACHE is None:
        _NC_CACHE = build_nc()
    in_maps = full_in_maps(x, w_qkv, b_qkv, w_proj)
    res = run_bass_kernel_spmd(_NC_CACHE, in_maps, core_ids=list(range(NCORES)))
    return gather(res.results, b_proj)


if __name__ == "__main__":
    rng = np.random.default_rng(0)
    x = rng.standard_normal((B, T, D), dtype=np.float32)
    w_qkv = rng.standard_normal((D, 3 * D), dtype=np.float32) / np.sqrt(D)
    b_qkv = np.zeros(3 * D, np.float32)
    w_proj = rng.standard_normal((D, D), dtype=np.float32) / np.sqrt(D)
    b_proj = np.zeros(D, np.float32)
    out = kernel(x, w_qkv, b_qkv, w_proj, b_proj)
    print(out.shape, out.dtype)



# revision 27
# speedup vs baseline: 1.0220x; 1.0067x over previous
"""Causal self-attention Trainium2 kernel.

Problem: B=4, T=2048, D=1024, H=16 heads (hd=64).
Sharding: 8 cores; core c -> batch c//2, heads (c%2)*8 .. +8.
Each core computes a partial output projection (its 512 rows of w_proj);
host sums the two partials per batch and adds b_proj.

Layout strategy (per core):
  - x^T [D, T] streamed in fp32, consumed as float32r (full-rate matmuls
    at near-fp32 precision for the QKV projections; host pre-transposed).
  - Q^T, K^T computed as [512, 2048] (head-dim on partitions) via
    W-stationary matmuls: out = W_chunk.T @ x^T, stored bf16.
  - V computed in natural [T, 512] layout (x^T-stationary), stored per-head
    augmented with a ones column -> [128k, head, 65], so the P@V matmul
    accumulates softmax denominators for free in row 64.
  - Scores computed transposed: S^T[k, q] = (K^T_chunk).T @ Q^T, causal
    blocks only; exp on ScalarE straight out of PSUM (no max subtraction --
    scaled scores are ~N(0,1), max << 88); triangular mask multiply only on
    diagonal 128-blocks.
  - P@V with V_aug stationary: out^T[65, q] accumulated over k-chunks in
    PSUM. Row 64 = sum of exp. Normalize with DVE reciprocal + GpSimd
    partition_broadcast; result written as A^T [512, 2048] bf16 which is
    exactly the lhsT needed for the output projection.
"""

import sys

for _p in ("/opt/trn_rl_repo",):
    if _p not in sys.path:
        sys.path.insert(0, _p)

import numpy as np
import ml_dtypes

import concourse.bass as bass
import concourse.mybir as mybir
import concourse.tile as tile
from concourse import bacc
from concourse.bass_utils import run_bass_kernel_spmd

BF16 = ml_dtypes.bfloat16

B, T, D = 4, 2048, 1024
H, HD = 16, 64
NCORES = 8
HPC = 8                  # heads per core
GCOLS = HPC * HD         # 512 columns of qkv per core per q/k/v
P = 128
NDC = D // P             # 8 contraction chunks of 128
NTT = T // P             # 16 t-tiles of 128
NQC = T // 512           # 4 q-chunks of 512
NMC = GCOLS // P         # 4 M-chunks per Q^T / K^T


def build_nc(trace_sim: bool = False):
    f32 = mybir.dt.float32
    f32r = mybir.dt.float32r
    bf16 = mybir.dt.bfloat16

    nc = bacc.Bacc("TRN2", target_bir_lowering=False, debug=False,
                   num_devices=NCORES)

    xT_d = nc.dram_tensor("xT", [D, T], f32r, kind="ExternalInput")
    wqk_d = nc.dram_tensor("wqk", [D, 2 * GCOLS], f32r, kind="ExternalInput")
    wv_d = nc.dram_tensor("wv", [D, GCOLS], f32r, kind="ExternalInput")
    wp_d = nc.dram_tensor("wp", [GCOLS, D], bf16, kind="ExternalInput")
    bqk_d = nc.dram_tensor("bqk", [P, 2 * NMC], f32, kind="ExternalInput")
    bv_d = nc.dram_tensor("bv", [GCOLS], f32, kind="ExternalInput")
    tri_d = nc.dram_tensor("tri", [P, P], bf16, kind="ExternalInput")
    out_d = nc.dram_tensor("outp", [T, D], f32, kind="ExternalOutput")

    with tile.TileContext(nc, trace_sim=trace_sim) as tc:
        with (
            tc.tile_pool(name="consts", bufs=1) as consts,
            tc.tile_pool(name="weights", bufs=1) as weights,
            tc.tile_pool(name="acts", bufs=1) as acts,
            tc.tile_pool(name="pt", bufs=3) as ptp,
            tc.tile_pool(name="norm", bufs=2) as normp,
            tc.tile_pool(name="outs", bufs=3) as outsp,
            tc.tile_pool(name="ps_mm", bufs=2, space="PSUM") as ps_mm,
            tc.tile_pool(name="ps_st", bufs=2, space="PSUM") as ps_st,
            tc.tile_pool(name="ps_o", bufs=2, space="PSUM") as ps_o,
        ):
            # ---------------- constants / weights ----------------
            tri_sb = consts.tile([P, P], bf16)
            nc.sync.dma_start(tri_sb[:], tri_d.ap())
            bqk_sb = consts.tile([P, 2 * NMC], f32)
            nc.sync.dma_start(bqk_sb[:], bqk_d.ap())
            # b_v replicated to all partitions (varies along free dim)
            bv_rep = consts.tile([P, GCOLS], f32)
            bv_ap = bv_d.ap()
            nc.gpsimd.dma_start(
                bv_rep[:],
                bass.AP(tensor=bv_ap.tensor, offset=bv_ap.offset,
                        ap=[[0, P]] + list(bv_ap.ap)),
            )

            # DMA order = first-use order: wv + x^T piece 0 feed the V and
            # mch-0 projections; then wqk, the remaining x^T pieces, wp.
            # First bytes on the wire: x^T piece 0 plus mch-0's K/Q weight
            # columns (K0 = cols 512:640, Q0 = cols 0:128) -- that is all the
            # first score matmuls need. wv follows for the P@V drain.
            wv_sb = weights.tile([P, NDC, GCOLS], f32r)
            xT_sb = acts.tile([P, NDC, T], f32r)
            wqk_sb = weights.tile([P, NDC, 2 * GCOLS], f32r)
            for dc in range(NDC):
                nc.sync.dma_start(
                    xT_sb[:, dc, 0:512], xT_d[dc * P:(dc + 1) * P, 0:512])
                nc.sync.dma_start(wqk_sb[:, dc, GCOLS:GCOLS + P],
                                  wqk_d[dc * P:(dc + 1) * P, GCOLS:GCOLS + P])
                nc.sync.dma_start(wqk_sb[:, dc, 0:P],
                                  wqk_d[dc * P:(dc + 1) * P, 0:P])
            for dc in range(NDC):
                nc.sync.dma_start(wv_sb[:, dc, :], wv_d[dc * P:(dc + 1) * P, :])
            for cp in range(1, NQC):
                for dc in range(NDC):
                    nc.sync.dma_start(
                        xT_sb[:, dc, cp * 512:(cp + 1) * 512],
                        xT_d[dc * P:(dc + 1) * P, cp * 512:(cp + 1) * 512])
            for dc in range(NDC):
                nc.sync.dma_start(wqk_sb[:, dc, P:GCOLS],
                                  wqk_d[dc * P:(dc + 1) * P, P:GCOLS])
                nc.sync.dma_start(wqk_sb[:, dc, GCOLS + P:],
                                  wqk_d[dc * P:(dc + 1) * P, GCOLS + P:])
            wp_sb = weights.tile([P, NMC, D], bf16)
            for hc in range(NMC):
                nc.sync.dma_start(wp_sb[:, hc, :], wp_d[hc * P:(hc + 1) * P, :])

            # ---------------- phases 1+2 interleaved ----------------
            # warm the ScalarE Exp table during the startup DMA window so
            # the first attention block doesn't pay the table load
            warm = consts.tile([1, 1], f32)
            nc.vector.memset(warm[:], 0.0)
            nc.scalar.activation(warm[:], warm[:],
                                 mybir.ActivationFunctionType.Exp)

            # V natural + ones column: [128, tt, head, 65]
            V_sb = acts.tile([P, NTT, HPC, HD + 1], bf16)
            nc.vector.memset(V_sb[:, :, :, HD], 1.0)

            def project_v(tts):
                for tt in tts:
                    pv = ps_mm.tile([P, 512], f32, tag="mm")
                    for dc in range(NDC):
                        nc.tensor.matmul(
                            pv[:],
                            xT_sb[:, dc, tt * P:(tt + 1) * P],
                            wv_sb[:, dc, :],
                            start=(dc == 0), stop=(dc == NDC - 1),
                        )
                    nc.vector.tensor_tensor(
                        V_sb[:, tt, :, 0:HD],
                        pv[:].rearrange("p (h d) -> p h d", h=HPC),
                        bv_rep[:].rearrange("p (h d) -> p h d", h=HPC),
                        mybir.AluOpType.add,
                    )

            # Q^T / K^T / A^T: [512, T] each, stored as [128, chunk, T].
            QT_sb = acts.tile([P, NMC, T], bf16)
            KT_sb = acts.tile([P, NMC, T], bf16)
            AT_sb = acts.tile([P, NMC, T], bf16)

            def project_qk(m, tc4s=range(NQC)):
                for tc4 in tc4s:
                    pq = ps_mm.tile([P, 512], f32, tag="mm")
                    for dc in range(NDC):
                        nc.tensor.matmul(
                            pq[:],
                            wqk_sb[:, dc, m * P:(m + 1) * P],
                            xT_sb[:, dc, tc4 * 512:(tc4 + 1) * 512],
                            start=(dc == 0), stop=(dc == NDC - 1),
                        )
                    dst = (QT_sb if m < NMC else KT_sb)
                    nc.vector.tensor_scalar_add(
                        dst[:, m % NMC, tc4 * 512:(tc4 + 1) * 512],
                        pq[:], bqk_sb[:, m:m + 1],
                    )

            def project_out(tts):
                for tt in tts:
                    for ncol in range(2):
                        pp = ps_mm.tile([P, 512], f32, tag="mm")
                        for hc in range(NMC):
                            nc.tensor.matmul(
                                pp[:],
                                AT_sb[:, hc, tt * P:(tt + 1) * P],
                                wp_sb[:, hc, ncol * 512:(ncol + 1) * 512],
                                start=(hc == 0), stop=(hc == NMC - 1),
                            )
                        ot = outsp.tile([P, 512], f32, tag="ot")
                        nc.vector.tensor_copy(ot[:], pp[:])
                        nc.sync.dma_start(
                            out_d[tt * P:(tt + 1) * P,
                                  ncol * 512:(ncol + 1) * 512],
                            ot[:],
                        )

            # Per 128-chunk: project K then Q, then both heads' attention.
            # The head pair sits at partitions 0-63 / 64-127, so the two
            # K=64 score matmuls auto-derive tile_position (0,0)/(64,0)
            # and can run concurrently on the two PE array row-halves.
            # V projection is smeared across mch 0's qc blocks (only V
            # k-tiles <= 4qc+3 are needed there); the output projection is
            # smeared across mch 3's qc blocks (proj of t-range qc needs
            # every head's qc block done).
            project_qk(NMC, [0])        # K chunk 0, columns for qc 0
            project_qk(0, [0])          # Q chunk 0, columns for qc 0
            project_v(range(0, 4))
            project_qk(NMC, [1, 2, 3])
            project_qk(0, [1, 2, 3])
            for mch in range(NMC):
                # last chunk: biggest qc block first so its projection
                # groups overlap the remaining attention blocks
                qc_order = range(NQC - 1, -1, -1) if mch == NMC - 1 \
                    else range(NQC)
                for qc in qc_order:
                    po = [ps_o.tile([HD + 1, 512], f32, tag="po",
                                    name=f"po{j}")
                          for j in range(2)]
                    nki = 4 * qc + 4
                    for ki in range(nki):
                        off = max(0, ki - 4 * qc) * P
                        # head-pair S^T into one 2-bank psum tile; one exp
                        # instruction covers both heads (halves the per-op
                        # PSUM-access overhead on ScalarE).
                        pshat = ps_st.tile([P, 2, 512], f32, tag="st")
                        pts = ptp.tile([P, 2, 512], bf16, tag="pt")
                        for j in range(2):
                            part = j * 64
                            nc.tensor.matmul(
                                pshat[:, j, off:512],
                                KT_sb[part:part + 64, mch,
                                      ki * P:(ki + 1) * P],
                                QT_sb[part:part + 64, mch,
                                      qc * 512 + off:(qc + 1) * 512],
                                start=True, stop=True,
                            )
                        nc.scalar.activation(
                            pts[:, :, off:512], pshat[:, :, off:512],
                            mybir.ActivationFunctionType.Exp,
                            scale=0.125,
                        )
                        if ki >= 4 * qc:
                            # diagonal block: zero out q < k entries
                            for j in range(2):
                                nc.vector.tensor_tensor(
                                    pts[:, j, off:off + P],
                                    pts[:, j, off:off + P],
                                    tri_sb[:], mybir.AluOpType.mult,
                                )
                        for j in range(2):
                            nc.tensor.matmul(
                                po[j][:, off:512],
                                V_sb[:, ki, 2 * mch + j, :],
                                pts[:, j, off:512],
                                start=(ki == 0), stop=(ki == nki - 1),
                            )
                    # normalize: row 64 of po is the softmax denominator.
                    # Copy the unnormalized block out first so the PSUM
                    # slot frees after two quick DVE ops; the broadcast +
                    # in-place multiply run off the critical path.
                    for j in range(2):
                        part = j * 64
                        rs = normp.tile([1, 512], f32, tag="rs")
                        nc.vector.reciprocal(rs[:], po[j][HD:HD + 1, :])
                        rep = normp.tile([64, 512], f32, tag="rep")
                        nc.gpsimd.partition_broadcast(rep[:], rs[0:1, :])
                        nc.vector.tensor_tensor(
                            AT_sb[part:part + 64, mch,
                                  qc * 512:(qc + 1) * 512],
                            po[j][0:HD, :], rep[:], mybir.AluOpType.mult,
                        )
                    if mch == 0 and qc < NQC - 1:
                        project_v(range(4 * (qc + 1), 4 * (qc + 2)))
                    if mch < NMC - 1:
                        # pipeline next chunk's K/Q projection (2 of the 8
                        # 512-col groups per qc block)
                        m_next = [NMC + mch + 1, mch + 1][qc // 2]
                        project_qk(m_next, [2 * qc % 4, 2 * qc % 4 + 1])
                    else:
                        project_out(range(4 * qc, 4 * (qc + 1)))

    nc.compile()
    return nc


def host_inputs(x, w_qkv, b_qkv):
    """Per-core input maps. Core c -> batch c//2, head group c%2."""
    x = np.asarray(x, np.float32)
    w_qkv = np.asarray(w_qkv, np.float32)
    b_qkv = np.asarray(b_qkv, np.float32)
    tri = (np.arange(P)[None, :] >= np.arange(P)[:, None]).astype(BF16)
    in_maps = []
    for c in range(NCORES):
        b, g = c // 2, c % 2
        cs = slice(g * GCOLS, (g + 1) * GCOLS)
        xT = np.ascontiguousarray(x[b].T).astype(np.float32)
        wqk = np.concatenate(
            [w_qkv[:, cs], w_qkv[:, D + g * GCOLS: D + (g + 1) * GCOLS]],
            axis=1).astype(np.float32)
        wv = np.ascontiguousarray(
            w_qkv[:, 2 * D + g * GCOLS: 2 * D + (g + 1) * GCOLS]).astype(np.float32)
        bq = b_qkv[cs].reshape(NMC, P).T
        bk = b_qkv[D + g * GCOLS: D + (g + 1) * GCOLS].reshape(NMC, P).T
        bqk = np.ascontiguousarray(
            np.concatenate([bq, bk], axis=1)).astype(np.float32)
        bv = np.ascontiguousarray(
            b_qkv[2 * D + g * GCOLS: 2 * D + (g + 1) * GCOLS]).astype(np.float32)
        in_maps.append({
            "xT": xT, "wqk": wqk, "wv": wv,
            "wp": None,  # filled by caller (needs w_proj)
            "bqk": bqk, "bv": bv, "tri": tri,
        })
    return in_maps


def full_in_maps(x, w_qkv, b_qkv, w_proj):
    w_proj = np.asarray(w_proj, np.float32)
    in_maps = host_inputs(x, w_qkv, b_qkv)
    for c in range(NCORES):
        g = c % 2
        in_maps[c]["wp"] = np.ascontiguousarray(
            w_proj[g * GCOLS:(g + 1) * GCOLS, :]).astype(BF16)
    return in_maps


def gather(results, b_proj):
    out = np.zeros((B, T, D), np.float32)
    for c in range(NCORES):
        out[c // 2] += results[c]["outp"]
    out += np.asarray(b_proj, np.float32)[None, None, :]
    return out


_NC_CACHE = None


def kernel(x, w_qkv, b_qkv, w_proj, b_proj):
    global _NC_CACHE
    if _NC_CACHE is None:
        _NC_CACHE = build_nc()
    in_maps = full_in_maps(x, w_qkv, b_qkv, w_proj)
    res = run_bass_kernel_spmd(_NC_CACHE, in_maps, core_ids=list(range(NCORES)))
    return gather(res.results, b_proj)


if __name__ == "__main__":
    rng = np.random.default_rng(0)
    x = rng.standard_normal((B, T, D), dtype=np.float32)
    w_qkv = rng.standard_normal((D, 3 * D), dtype=np.float32) / np.sqrt(D)
    b_qkv = np.zeros(3 * D, np.float32)
    w_proj = rng.standard_normal((D, D), dtype=np.float32) / np.sqrt(D)
    b_proj = np.zeros(D, np.float32)
    out = kernel(x, w_qkv, b_qkv, w_proj, b_proj)
    print(out.shape, out.dtype)

